# revision 3
# baseline (speedup 1.0000x reference)
"""Cross-attention kernel for 8 Trainium2 NeuronCores — v2.

Problem (hardcoded): B=2, NQ=NKV=2048, QDIM=KVDIM=1024, H=16, HD=64.

Sharding: tensor-parallel over heads — 2 heads per core. Each core computes
its heads' Q/K/V projections, scores, softmax and context for the full
sequence; AllToAll reshards context from head-split to token-split so the
output projection is fully local. Output tokens are interleaved across
batches (core j owns b0 tokens [j*256,(j+1)*256) and the same-index b1
tokens), which lets the AllToAll split in two: the batch-0 half runs —
and its output projection — overlapped with batch-1 attention.

Engine plan per kt-pair "round" (the schedule is ACT-bound):
  PE : probs@V as ONE fp8e4 DoubleRow matmul per head (contraction 256 over
       the kt-pair), scores for the NEXT pair (bf16, two heads row-tiled on
       the PE array), plus an ~850ns chunk of projection / output-projection
       work threaded into the round's slack.
  ACT: two wide exp's (N=1024: both heads of one kt), fp8 output.
  DVE/Pool: PSUM->SBUF copies, softmax normalize (reciprocal + partition
       broadcast + multiply); the PV ones-column yields denominators free.
"""

import numpy as np
import ml_dtypes

import concourse.bass as bass
import concourse.mybir as mybir
import concourse.tile as tile
from concourse import bacc
from concourse.bass_utils import run_bass_kernel_spmd

N_CORES = 8
B = 2
NQ = NKV = 2048
C = 1024          # model dim (QDIM=KVDIM=INNER)
H, HD = 16, 64
T = B * NQ        # 4096 flattened tokens
DL = 128          # local head dims per core (2 heads * 64)
TSH = T // N_CORES  # 512 output tokens per core
HALF = TSH // 2   # 256 per batch
SCALE = HD ** -0.5

F32 = mybir.dt.float32
BF16 = mybir.dt.bfloat16
FP8 = mybir.dt.float8e4

_NC_CACHE = None
_LAST_RESULTS = None


def _build(with_collective=True, reps=None, stop_after=None):
    nc = bacc.Bacc("TRN2", target_bir_lowering=False, debug=False,
                   num_devices=N_CORES)

    qT = nc.dram_tensor("qT", [C, T], BF16, kind="ExternalInput")
    kvT = nc.dram_tensor("kvT", [C, T], BF16, kind="ExternalInput")
    wq = nc.dram_tensor("wq", [C, DL], BF16, kind="ExternalInput")
    wk = nc.dram_tensor("wk", [C, DL], BF16, kind="ExternalInput")
    wv = nc.dram_tensor("wv", [C, DL], BF16, kind="ExternalInput")
    wo = nc.dram_tensor("wo", [C, C], BF16, kind="ExternalInput")
    bias = nc.dram_tensor("bias", [C], F32, kind="ExternalInput")
    out = nc.dram_tensor("out", [TSH, C], BF16, kind="ExternalOutput")

    CC = C // 128   # 8 contraction chunks
    NR = 64         # rounds: 8 groups x 8 kt-pairs
    Exp = mybir.ActivationFunctionType.Exp
    DR = mybir.MatmulPerfMode.DoubleRow

    with tile.TileContext(nc) as tc:
        with (
            tc.tile_pool(name="consts", bufs=1) as consts,
            tc.tile_pool(name="xt", bufs=3) as xt,
            tc.tile_pool(name="probs", bufs=4) as probs_p,
            tc.tile_pool(name="norm", bufs=2) as norm,
            tc.tile_pool(name="outp", bufs=2) as outp,
            tc.tile_pool(name="dram", bufs=1, space="DRAM") as dram,
        ):
            # ---- persistent SBUF ----
            wq_sb = consts.tile([128, CC, DL], BF16)
            wk_sb = consts.tile([128, CC, DL], BF16)
            wv_sb = consts.tile([128, CC, DL], BF16)
            wo_sb = consts.tile([128, CC, C], BF16)
            bias_sb = consts.tile([128, C], F32)
            Kd_sb = consts.tile([128, T], BF16)   # K^T: [d_local, token]
            Qd_sb = consts.tile([128, T], BF16)   # Q^T: [d_local, token]
            # V natural [token, d] fp8, 32 tiles of [128, 144]:
            # head h at cols h*72 .. h*72+63, ones col at h*72+64 (the kt
            # stride of 144 B keeps DoubleRow's step%16==0 constraint)
            V_sb = consts.tile([128, T // 128, 144], FP8)

            # const DMAs: the sync queue carries what the prologue needs, in
            # need-order (emitted inside xdma(0)); wo chunks ride the scalar
            # queue so nothing big blocks the first x-tiles
            wq_r = wq.ap().rearrange("(n p) d -> p n d", p=128)
            wk_r = wk.ap().rearrange("(n p) d -> p n d", p=128)
            wv_r = wv.ap().rearrange("(n p) d -> p n d", p=128)
            wo_r = wo.ap().rearrange("(n p) e -> p n e", p=128)
            nc.vector.memset(V_sb[:, :, 64:65], 1.0)
            nc.vector.memset(V_sb[:, :, 136:137], 1.0)

            qT_r = qT.ap().rearrange("(n p) t -> p n t", p=128)
            kvT_r = kvT.ap().rearrange("(n p) t -> p n t", p=128)

            def _body(_it=None):
                # PSUM: pair(2 banks x2) + psc(2 banks) + pp(1 bank x2) = 8
                with tc.tile_pool(name="ps", bufs=1, space="PSUM") as ps:
                    st = {}  # live tiles: kvt/qt per t-tile, pr per pair, psc

                    a2a_inA = dram.tile([N_CORES, DL, HALF], BF16)
                    a2a_inB = dram.tile([N_CORES, DL, HALF], BF16)
                    if with_collective:
                        a2a_outA = dram.tile([N_CORES, DL, HALF], BF16)
                        a2a_outB = dram.tile([N_CORES, DL, HALF], BF16)
                    else:
                        a2a_outA, a2a_outB = a2a_inA, a2a_inB

                    def xdma(tt):
                        t0 = tt * 512
                        kvt = xt.tile([128, CC, 512], BF16, tag="kvt", name=f"kvt{tt}")
                        qt_ = xt.tile([128, CC, 512], BF16, tag="qt", name=f"qt{tt}")
                        st[f"x{tt}"] = (kvt, qt_)
                        if tt == 0:
                            # prologue need-order: wk -> kvt0 -> qt0 -> wq -> wv
                            nc.sync.dma_start(out=wk_sb, in_=wk_r)
                            nc.sync.dma_start(out=kvt, in_=kvT_r[:, :, t0:t0 + 512])
                            nc.sync.dma_start(out=qt_, in_=qT_r[:, :, t0:t0 + 512])
                            nc.sync.dma_start(out=wq_sb, in_=wq_r)
                            nc.sync.dma_start(out=wv_sb, in_=wv_r)
                        else:
                            nc.sync.dma_start(out=kvt, in_=kvT_r[:, :, t0:t0 + 512])
                            nc.sync.dma_start(out=qt_, in_=qT_r[:, :, t0:t0 + 512])

                    # ---- projection chunks (6 per t-tile, ~850ns PE each) ----
                    def vproj_half(tt, kvt, lo):
                        psv = st["psv"]
                        for s4 in (lo, lo + 1):
                            for cc in range(CC):
                                nc.tensor.matmul(
                                    psv[:, s4, :],
                                    lhsT=kvt[:, cc, s4 * 128:(s4 + 1) * 128],
                                    rhs=wv_sb[:, cc, :],
                                    start=(cc == 0), stop=(cc == CC - 1))
                        # psv [128, (2 s4, 2 h, 64)] -> V_sb [128, (2 ti, 2 h@72, 64)]
                        vdst = V_sb[:, tt * 4 + lo, 0:64]
                        vdst = bass.AP(tensor=vdst.tensor, offset=vdst.offset,
                                       ap=[vdst.ap[0], [144, 2], [72, 2], [1, 64]])
                        nc.vector.tensor_copy(
                            out=vdst,
                            in_=psv[:, lo:lo + 2].rearrange("p s (g x) -> p s g x", g=2))

                    def proj_chunk(tt, c):
                        t0 = tt * 512
                        kvt, qt_ = st[f"x{tt}"]
                        if c == 0:
                            if tt + 1 < 8:
                                xdma(tt + 1)
                            psk = ps.tile([128, 512], F32, tag="pp", bufs=2,
                                          name="psk")
                            st["psk"] = psk
                            for cc in range(4):
                                nc.tensor.matmul(psk, lhsT=wk_sb[:, cc, :],
                                                 rhs=kvt[:, cc, :],
                                                 start=(cc == 0), stop=False)
                        elif c == 1:
                            psk = st["psk"]
                            for cc in range(4, CC):
                                nc.tensor.matmul(psk, lhsT=wk_sb[:, cc, :],
                                                 rhs=kvt[:, cc, :],
                                                 start=False, stop=(cc == CC - 1))
                            nc.vector.tensor_copy(out=Kd_sb[:, t0:t0 + 512], in_=psk)
                        elif c == 2:
                            psq = ps.tile([128, 512], F32, tag="pp", bufs=2, name="psq")
                            st["psq"] = psq
                            for cc in range(4):
                                nc.tensor.matmul(psq, lhsT=wq_sb[:, cc, :],
                                                 rhs=qt_[:, cc, :],
                                                 start=(cc == 0), stop=False)
                        elif c == 3:
                            psq = st["psq"]
                            for cc in range(4, CC):
                                nc.tensor.matmul(psq, lhsT=wq_sb[:, cc, :],
                                                 rhs=qt_[:, cc, :],
                                                 start=False, stop=(cc == CC - 1))
                            nc.vector.tensor_copy(out=Qd_sb[:, t0:t0 + 512], in_=psq)
                        elif c == 4:
                            st["psv"] = ps.tile([128, 4, 128], F32, tag="pp", bufs=2,
                                                name="psv")
                            vproj_half(tt, kvt, 0)
                        elif c == 5:
                            vproj_half(tt, kvt, 2)

                    # ---- attention ----
                    def scores_exp(p):
                        g, _ = divmod(p, 8)
                        b, qv = divmod(g, 4)
                        q0 = b * NQ + qv * 512
                        pr = probs_p.tile([128, 2, 2, 512], FP8, tag="pr",
                                          name=f"pr{p}")
                        st[f"pr{p % 4}"] = pr
                        for j, kt in enumerate((2 * (p % 8), 2 * (p % 8) + 1)):
                            k0 = b * NKV + kt * 128
                            pairT = ps.tile([128, 1024], F32, tag="pair", bufs=2,
                                            name="pair")
                            for h in range(2):
                                hs = slice(h * 64, (h + 1) * 64)
                                nc.tensor.matmul(
                                    pairT[:, h * 512:(h + 1) * 512],
                                    lhsT=Kd_sb[hs, k0:k0 + 128],
                                    rhs=Qd_sb[hs, q0:q0 + 512],
                                    start=True, stop=True)
                            nc.scalar.activation(
                                out=pr[:, j, :, :],
                                in_=pairT[:].rearrange("p (h q) -> p h q", h=2),
                                func=Exp, scale=SCALE)

                    def pv(p):
                        g, r = divmod(p, 8)
                        b = g // 4
                        if r == 0:
                            st["psc"] = ps.tile([65, 1024], F32, tag="psc",
                                                bufs=1, name="psc")
                        psc = st["psc"]
                        pr = st[f"pr{p % 4}"]
                        ti0 = b * 16 + 2 * r
                        for h in range(2):
                            nc.tensor.matmul(
                                psc[:, h * 512:(h + 1) * 512],
                                lhsT=V_sb[:, ti0:ti0 + 2, h * 72:h * 72 + 65],
                                rhs=pr[:, :, h, :],
                                perf_mode=DR,
                                start=(r == 0), stop=(r == 7))

                    def norm_out(g):
                        b, qv = divmod(g, 4)
                        psc = st["psc"]
                        if g < 7:
                            # copy-out releases the PSUM accumulator after one
                            # DVE op; the reciprocal/broadcast/multiply chain
                            # then runs off the critical path from SBUF
                            ctxS = norm.tile([65, 1024], F32, tag="ctxS",
                                             name="ctxS")
                            nc.vector.tensor_copy(out=ctxS, in_=psc)
                        else:
                            ctxS = psc  # last group: nothing follows, skip copy
                        recip = norm.tile([1, 1024], F32, tag="recip", name="recip")
                        nc.vector.reciprocal(out=recip, in_=ctxS[64:65, :])
                        bc = norm.tile([64, 1024], F32, tag="bc", name="bc")
                        nc.gpsimd.partition_broadcast(bc[:], recip[:])
                        # ctxn stored s-major: [d, (s, h, t)] so the a2a write
                        # is a single DMA whose (s,h) dims merge on both sides
                        ctxn = norm.tile([64, 2, 2, HALF], BF16, tag="ctxn",
                                         name="ctxn")
                        ctxn_hst = bass.AP(
                            tensor=ctxn.tensor, offset=ctxn.offset,
                            ap=[ctxn.ap[0], [HALF, 2], [2 * HALF, 2], [1, HALF]])
                        nc.vector.tensor_mul(
                            ctxn_hst,
                            ctxS[0:64, :].rearrange("p (h s t) -> p h s t",
                                                    h=2, s=2),
                            bc[:].rearrange("p (h s t) -> p h s t", h=2, s=2))
                        a2a = a2a_inA if b == 0 else a2a_inB
                        sl = a2a[2 * qv, 0:64, :]
                        dst = bass.AP(
                            tensor=sl.tensor, offset=sl.offset,
                            ap=[sl.ap[0], [DL * HALF, 2], [64 * HALF, 2],
                                [1, HALF]])
                        nc.sync.dma_start(out=dst, in_=ctxn[:])

                    # ---- output projection (local tokens; A=b0, B=b1) ----
                    def ctxf_load(which):
                        src = a2a_outA if which == 0 else a2a_outB
                        ctxf = outp.tile([128, N_CORES, HALF], BF16,
                                         tag=f"ctxf{which}", name=f"ctxf{which}")
                        st[f"ctxf{which}"] = ctxf
                        nc.sync.dma_start(
                            out=ctxf, in_=src[:].rearrange("i p t -> p i t"))

                    def oproj_chunk(which, m, half, c):
                        ctxf = st[f"ctxf{which}"]
                        if c == 0:
                            pso = ps.tile([128, 512], F32, tag="pp", bufs=2,
                                          name="pso")
                            st["pso"] = pso
                            lo = 0
                        else:
                            pso = st["pso"]
                            lo = 4
                        for i in range(lo, lo + 4):
                            nc.tensor.matmul(
                                pso, lhsT=ctxf[:, i, m * 128:(m + 1) * 128],
                                rhs=wo_sb[:, i, half * 512:(half + 1) * 512],
                                start=(i == 0), stop=(i == N_CORES - 1))
                        if c == 1:
                            if half == 0:
                                st["ob"] = outp.tile([128, C], BF16, tag="ob",
                                                     name="ob")
                            ob = st["ob"]
                            nc.vector.tensor_add(
                                ob[:, half * 512:(half + 1) * 512], pso,
                                bias_sb[:, half * 512:(half + 1) * 512])
                            if half == 1:
                                nc.sync.dma_start(
                                    out=out.ap()[which * HALF + m * 128:
                                                 which * HALF + (m + 1) * 128, :],
                                    in_=ob)

                    def collective(which):
                        if not with_collective:
                            return
                        a_in = a2a_inA if which == 0 else a2a_inB
                        a_out = a2a_outA if which == 0 else a2a_outB
                        nc.gpsimd.collective_compute(
                            "AllToAll", mybir.AluOpType.bypass,
                            replica_groups=[list(range(N_CORES))],
                            ins=[a_in.opt()], outs=[a_out.opt()])

                    # ---- schedule ----
                    # extra-work chunks keyed by (round, phase): phase 0 =
                    # before the lookahead scores, 1 = after pv.
                    # Deadlines (lookahead-1): K of tile (2p+1)//4 before
                    # round p-1; V half before its pair's pv; Q of a group's
                    # tile before the group's first scores.
                    extras = {}

                    def add(r, ph, fn):
                        extras.setdefault((r, ph), []).append(fn)

                    def addc(r, ph, tt, c):
                        add(r, ph, lambda: proj_chunk(tt, c))

                    # b0 proj tiles: K (c0,c1), V (c4,c5) on their score/pv
                    # deadlines; Q (c2,c3) deferred to just before its group
                    addc(0, 0, 0, 4)
                    addc(0, 1, 0, 5); addc(0, 1, 1, 0)
                    addc(1, 0, 1, 1); addc(1, 1, 1, 4)
                    addc(2, 0, 1, 5); addc(2, 1, 2, 0)
                    addc(3, 0, 2, 1); addc(3, 1, 2, 4)
                    addc(4, 0, 2, 5); addc(4, 1, 3, 0)
                    addc(5, 0, 3, 1); addc(5, 1, 3, 4); addc(5, 1, 1, 2)
                    addc(6, 0, 3, 5); addc(6, 1, 1, 3)
                    addc(11, 0, 2, 2); addc(12, 0, 2, 3)
                    addc(19, 0, 3, 2); addc(20, 0, 3, 3)
                    # b1 tiles 4-7: K/V chunks one per round over rounds
                    # 10..25; Q chunks trail, each just before its group
                    b1kv = [(tt, c) for tt in range(4, 8) for c in (0, 1, 4, 5)]
                    for k, (tt, c) in enumerate(b1kv):
                        addc(10 + k, 1, tt, c)
                    addc(26, 1, 4, 2); addc(27, 1, 4, 3)
                    addc(33, 1, 5, 2); addc(34, 1, 5, 3)
                    addc(41, 1, 6, 2); addc(42, 1, 6, 3)
                    addc(49, 1, 7, 2); addc(50, 1, 7, 3)
                    # collective A fires after group 3's norm (round 32)
                    add(32, 1, lambda: collective(0))
                    add(33, 0, lambda: ctxf_load(0))
                    # out-proj A spread over rounds clear of other extras
                    for k, (m, half, c) in enumerate(
                            (m, h, c) for m in range(2) for h in range(2)
                            for c in range(2)):
                        add((36, 37, 38, 39, 44, 45, 46, 47)[k], 1,
                            (lambda m=m, h=half, c=c: oproj_chunk(0, m, h, c)))

                    def run_extras(r, ph):
                        for fn in extras.get((r, ph), ()):
                            fn()

                    # ---- prologue: project tile 0, score+exp pair 0 ----
                    xdma(0)
                    for c in range(4):
                        proj_chunk(0, c)
                    scores_exp(0)
                    # wo/bias ride the gpsimd SWDGE path: Pool is idle and
                    # these transfers (needed from round 40 only) must not
                    # contend with the prologue's kvt/qt loads
                    for i in range(CC):
                        nc.gpsimd.dma_start(out=wo_sb[:, i, :], in_=wo_r[:, i, :])
                    bias_bc = bass.AP(tensor=bias, offset=0, ap=[[0, 128], [1, C]])
                    nc.gpsimd.dma_start(out=bias_sb[:], in_=bias_bc)
                    if stop_after == "proj":
                        return

                    # ---- rounds (pv lags scores by one round so a group's
                    # first pv never head-of-line-blocks the score stream
                    # while the previous group's accumulator is copied out) --
                    for R in range(NR):
                        run_extras(R, 0)
                        if R + 1 < NR:
                            scores_exp(R + 1)
                        if R >= 1:
                            pv(R - 1)
                        if R >= 8 and R % 8 == 0:
                            norm_out(R // 8 - 1)
                        if R == NR - 1:
                            pv(R)
                            norm_out(7)
                        run_extras(R, 1)
                    if stop_after == "attn":
                        return

                    # ---- tail: collective B + out-proj B ----
                    collective(1)
                    ctxf_load(1)
                    # keep-warm: the PE idles ~10us on the norm/a2a round-trip
                    # before out-proj B; back-to-back throwaway matmuls keep
                    # the HAM clock gate open so out-proj runs at full rate
                    warm = ps.tile([128, 1024], F32, tag="pair", bufs=2,
                                   name="warm")
                    for w in range(44):
                        nc.tensor.matmul(warm[:, 0:512],
                                         lhsT=Kd_sb[0:64, 0:128],
                                         rhs=Qd_sb[0:64, 0:512],
                                         start=True, stop=True)
                    for m in range(2):
                        for half in range(2):
                            for c in range(2):
                                oproj_chunk(1, m, half, c)

            if reps is None:
                _body()
            else:
                with tc.For_i(0, reps, 1) as _it:
                    _body(_it)
    nc.compile()
    return nc


def _get_nc():
    global _NC_CACHE
    if _NC_CACHE is None:
        _NC_CACHE = _build()
    return _NC_CACHE


def prep_in_maps(query, key_value, w_q, w_kv, w_out, b_out):
    bf = ml_dtypes.bfloat16
    f8 = ml_dtypes.float8_e4m3
    q2 = np.asarray(query, np.float32).reshape(T, C)
    kv2 = np.asarray(key_value, np.float32).reshape(T, C)
    qT = np.ascontiguousarray(q2.T).astype(bf)
    kvT = np.ascontiguousarray(kv2.T).astype(bf)
    wo = np.asarray(w_out, np.float32).astype(bf)
    bias = np.asarray(b_out, np.float32)

    in_maps = []
    for j in range(N_CORES):
        cs = slice(j * DL, (j + 1) * DL)
        in_maps.append({
            "qT": qT,
            "kvT": kvT,
            "wq": np.ascontiguousarray(np.asarray(w_q, np.float32)[:, cs]).astype(bf),
            "wk": np.ascontiguousarray(np.asarray(w_kv, np.float32)[:, cs]).astype(bf),
            "wv": np.ascontiguousarray(
                np.asarray(w_kv, np.float32)[:, C + j * DL: C + (j + 1) * DL]).astype(bf),
            "wo": wo,
            "bias": bias,
        })
    return in_maps


def kernel(query, key_value, w_q, w_kv, w_out, b_out):
    global _LAST_RESULTS
    in_maps = prep_in_maps(query, key_value, w_q, w_kv, w_out, b_out)
    nc = _get_nc()
    res = run_bass_kernel_spmd(nc, in_maps, core_ids=list(range(N_CORES)))
    _LAST_RESULTS = res
    # core j's out rows: [0:256) = batch-0 tokens [j*256,(j+1)*256),
    # [256:512) = batch-1 tokens at the same index
    full = np.empty((T, C), np.float32)
    for j in range(N_CORES):
        o = np.asarray(res.results[j]["out"], dtype=np.float32)
        full[j * HALF:(j + 1) * HALF] = o[:HALF]
        full[NQ + j * HALF:NQ + (j + 1) * HALF] = o[HALF:]
    return full.reshape(B, NQ, C)
